# revision 1
# baseline (speedup 1.0000x reference)
"""DeepSeekMoE forward on 8 Trainium2 NeuronCores.

Sharding: expert-parallel. Core c owns expert group c (8 of 64 experts) and a
1/8 column slice of the shared expert. The gate is replicated; its expert axis
is permuted per-core (own group first) so all cores run one SPMD program.
Each core produces a partial-sum [T, H]; the host reduces the 8 partials.

The gate runs in fp32 (expert selection gaps are ~1e-5; bf16 would flip
picks); expert/shared matmuls run in bf16 with fp32 accumulation. Dispatch is
capacity-128 and fully matmul-based: per-expert token slots come from exact
integer prefix-sum matmuls against a triangular matrix, giving one-hot
slot matrices Me [token, slot] per (expert, chunk). Token gather is
x_chunk^T @ Me on the tensor engine (yields X_e^T directly), and the combine
is Me^T @ y_e accumulated in PSUM over the local experts - no indirect DMA
anywhere (SWDGE descriptor generation for row gathers was the bottleneck).

Measured on the 8 axon-tunneled NC_v3 cores: ~55-77us marginal per kernel
body (repeat-N wall differencing; at the 24MB/core bf16 weight-streaming DMA
floor), ~0.9-1.1e5 ns single-shot estimate incl. resident loads and the Tile
tail barrier; absmax relative error 3.6e-3 vs the fp32 CPU reference (pure
bf16 expert/shared quantization noise; expert selection matches exactly).
"""
import sys

sys.path.insert(0, "/opt/trn_rl_repo")

import numpy as np
import ml_dtypes
import orjson

import concourse.bass as bass
import concourse.mybir as mybir
from concourse.tile import TileContext
from concourse.masks import make_identity
from concourse.bass_utils import run_bass_kernel_spmd

F32 = mybir.dt.float32
BF16 = mybir.dt.bfloat16
BF = ml_dtypes.bfloat16

P = 128          # partitions / token chunk / capacity
T = 1024         # tokens
H = 1024         # hidden
II = 512         # expert intermediate
E = 64           # routed experts
EL = 8           # local experts per core
NC = 8           # cores
C = 128          # per-expert token capacity
NCH = T // P     # token chunks
KH = H // P      # contraction chunks over H
STK = 16         # extraction stack columns: whi[8] wlo[8]
EXBUFS = 2       # per-expert ring depth; 3 PRODUCES WRONG RESULTS (relerr 1.5)
EGRP = 4         # experts per combine pass


def _split_waits_json(bir_bytes: bytes, max_waits: int = 1) -> bytes:
    """This walrus build accepts at most one sync wait per instruction; hoist
    extras into standalone EventSemaphore instructions on the same engine."""
    d = orjson.loads(bir_bytes)
    for fn in d.get("functions", []):
        for blk in fn.get("blocks", []):
            out = []
            for inst in blk.get("instructions", []):
                si = inst.get("sync_info") or {}
                waits = si.get("on_wait") or []
                if len(waits) > max_waits:
                    for j, w in enumerate(waits[:-max_waits]):
                        out.append({
                            "debug": inst.get("debug", 0),
                            "engine": inst["engine"],
                            "ins": [], "outs": [],
                            "name": f"{inst['name']}_hw{j}",
                            "opcode": "EventSemaphore",
                            "sync_info": {"on_update": [], "on_wait": [w]},
                        })
                    si["on_wait"] = waits[-max_waits:]
                    inst["sync_info"] = si
                out.append(inst)
            blk["instructions"] = out
    return orjson.dumps(d)


def _build_program(repeat=1):
    nc = bass.Bass("TRN2")
    AF = mybir.ActivationFunctionType

    # ---- I/O ----
    xtf_in = nc.dram_tensor("xtf", [P, KH * T], F32, kind="ExternalInput")
    xthi_in = nc.dram_tensor("xthi", [P, KH * T], BF16, kind="ExternalInput")
    xloc_in = nc.dram_tensor("xloc", [P, NCH * H], BF16, kind="ExternalInput")
    gtf_in = nc.dram_tensor("gtf", [P, KH * E], F32, kind="ExternalInput")
    wa_in = nc.dram_tensor("wbloba", [EL, P, 8192], BF16, kind="ExternalInput")
    wd_in = nc.dram_tensor("wblobd", [EL, P, 4096], BF16, kind="ExternalInput")
    shw_in = nc.dram_tensor("shw", [P, 3072], BF16, kind="ExternalInput")
    ltri_in = nc.dram_tensor("ltri", [P, P], F32, kind="ExternalInput")
    r127_in = nc.dram_tensor("r127", [P, P], F32, kind="ExternalInput")
    iotac_in = nc.dram_tensor("iotac", [P, C], F32, kind="ExternalInput")
    bias_in = nc.dram_tensor("biasbc", [P, E], F32, kind="ExternalInput")
    out_d = nc.dram_tensor("out", [T, H], F32, kind="ExternalOutput")

    with TileContext(nc) as tc:
        with tc.tile_pool(name="cst", bufs=1) as cst, \
             tc.tile_pool(name="big", bufs=1) as big, \
             tc.tile_pool(name="wts", bufs=2) as wts, \
             tc.tile_pool(name="rt", bufs=2) as rt, \
             tc.tile_pool(name="ex", bufs=EXBUFS) as ex, \
             tc.tile_pool(name="cmb", bufs=EGRP) as cmb, \
             tc.tile_pool(name="ppA", bufs=1, space="PSUM") as ppA, \
             tc.tile_pool(name="ppB", bufs=2, space="PSUM") as ppB:

            # ---- resident loads ----
            xtf = big.tile([P, KH * T], F32)
            nc.sync.dma_start(xtf[:], xtf_in[:])
            xthi = big.tile([P, KH * T], BF16)
            nc.sync.dma_start(xthi[:], xthi_in[:])
            xloc = big.tile([P, NCH * H], BF16)
            nc.sync.dma_start(xloc[:], xloc_in[:])
            gtf = cst.tile([P, KH * E], F32)
            nc.sync.dma_start(gtf[:], gtf_in[:])
            shw = cst.tile([P, 3072], BF16)
            nc.sync.dma_start(shw[:], shw_in[:])
            ltri = cst.tile([P, P], F32)
            nc.sync.dma_start(ltri[:], ltri_in[:])
            r127 = cst.tile([P, P], F32)
            nc.sync.dma_start(r127[:], r127_in[:])
            iotac = cst.tile([P, C], F32)
            nc.sync.dma_start(iotac[:], iotac_in[:])
            biasbc = cst.tile([P, E], F32)
            nc.sync.dma_start(biasbc[:], bias_in[:])
            ident = cst.tile([P, P], BF16)
            make_identity(nc, ident[:])

            consts = (xtf, xthi, xloc, gtf, shw, ltri, r127, iotac, biasbc,
                      ident)
            pools = (wts, rt, ex, cmb, ppA, ppB)
            for rep in range(repeat):
                _phase_body(nc, AF, rep, consts, (wa_in, wd_in, out_d), big, pools)

    orig = nc.to_json_bytes
    nc.to_json_bytes = lambda: _split_waits_json(orig())
    return nc


def _phase_body(nc, AF, rep, consts, drams, big, pools):
    (xtf, xthi, xloc, gtf, shw, ltri, r127, iotac, biasbc, ident) = consts
    (wa_in, wd_in, out_d) = drams
    (wts, rt, ex, cmb, ppA, ppB) = pools

    # expert weight blobs: (eg|eu) and (ed) prefetched in separate rings so
    # layer 1 starts as soon as its half lands and slots free after last use
    wa = [wts.tile([P, 8192], BF16, tag="wexpa", name=f"wa{rep}_{i}")
          for i in range(EL)]
    wd = [wts.tile([P, 4096], BF16, tag="wexpd", name=f"wd{rep}_{i}")
          for i in range(EL)]
    for e in range(EL):
        nc.sync.dma_start(wa[e][:], wa_in[e, :, :])
        nc.sync.dma_start(wd[e][:], wd_in[e, :, :])

    # ---- phase S: shared expert (column slice) -> yshf staging ----
    h2sh = big.tile([P, T], BF16, tag="h2sh", name=f"h2sh{rep}")
    for th in range(2):
        pg = ppA.tile([P, 512], F32, tag="l1g")
        pu = ppA.tile([P, 512], F32, tag="l1u")
        for kk in range(KH):
            xs = xthi[:, kk * T + th * 512: kk * T + (th + 1) * 512]
            nc.tensor.matmul(pg[:], lhsT=shw[:, kk * P:(kk + 1) * P],
                             rhs=xs, start=(kk == 0), stop=(kk == KH - 1))
        for kk in range(KH):
            xs = xthi[:, kk * T + th * 512: kk * T + (th + 1) * 512]
            nc.tensor.matmul(
                pu[:], lhsT=shw[:, 1024 + kk * P: 1024 + (kk + 1) * P],
                rhs=xs, start=(kk == 0), stop=(kk == KH - 1))
        sa = rt.tile([P, 512], F32, tag="shact")
        nc.scalar.activation(sa[:], pg[:], AF.Silu)
        nc.vector.tensor_mul(h2sh[:, th * 512:(th + 1) * 512], sa[:], pu[:])
    yshf = big.tile([P, NCH * H], F32, tag="yshf", name=f"yshf{rep}")
    for tch in range(NCH):
        for hh in range(2):
            yp = ppB.tile([P, 512], F32, tag="l3")
            nc.tensor.matmul(
                yp[:], lhsT=h2sh[:, tch * P:(tch + 1) * P],
                rhs=shw[:, 2048 + hh * 512: 2048 + (hh + 1) * 512],
                start=True, stop=True)
            nc.scalar.activation(
                yshf[:, tch * H + hh * 512: tch * H + (hh + 1) * 512],
                yp[:], AF.Copy)

    # ---- phase R: routing (replicated on every core) ----
    slotbuf = big.tile([P, NCH * E], F32, tag="slotbuf", name=f"slotbuf{rep}")
    stk = big.tile([P, NCH * STK], BF16, tag="stk", name=f"stk{rep}")
    runoff = big.tile([P, E], F32, tag="runoff", name=f"runoff{rep}")
    nc.vector.memset(runoff[:], 0.0)

    for ch in range(NCH):
        lg = ppB.tile([P, E], F32, tag="small")
        for kk in range(KH):
            nc.tensor.matmul(
                lg[:], lhsT=xtf[:, kk * T + ch * P: kk * T + ch * P + P],
                rhs=gtf[:, kk * E:(kk + 1) * E],
                start=(kk == 0), stop=(kk == KH - 1))
        # scores = sigmoid(logits) + bias
        sig = rt.tile([P, E], F32, tag="sig")
        nc.scalar.activation(sig[:], lg[:], AF.Sigmoid)
        nc.vector.tensor_add(sig[:], sig[:], biasbc[:])
        # group top-4 mask
        gmax = rt.tile([P, 8], F32, tag="gmax")
        nc.vector.tensor_reduce(
            out=gmax[:], in_=sig[:].rearrange("p (g e) -> p g e", e=8),
            op=mybir.AluOpType.max, axis=mybir.AxisListType.X)
        t8g = rt.tile([P, 8], F32, tag="t8g")
        nc.vector.max(out=t8g[:], in_=gmax[:])
        gmask = rt.tile([P, 8], F32, tag="gmask")
        nc.vector.tensor_scalar(gmask[:], gmax[:], t8g[:, 3:4], None,
                                op0=mybir.AluOpType.is_ge)
        gmx = rt.tile([P, E], F32, tag="gmx")
        nc.vector.tensor_copy(gmx[:], gmask[:].unsqueeze(2)
                              .to_broadcast([P, 8, 8]))
        # masked scores, top-6 mask
        msc = rt.tile([P, E], F32, tag="msc")
        nc.vector.tensor_mul(msc[:], sig[:], gmx[:])
        t8e = rt.tile([P, 8], F32, tag="t8e")
        nc.vector.max(out=t8e[:], in_=msc[:])
        m6 = rt.tile([P, E], F32, tag="m6")
        nc.vector.tensor_scalar(m6[:], msc[:], t8e[:, 5:6], None,
                                op0=mybir.AluOpType.is_ge)
        # normalized combine weights for the 8 local experts, split hi+lo
        cu = rt.tile([P, E], F32, tag="cu")
        nc.vector.tensor_mul(cu[:], msc[:], m6[:])
        den = rt.tile([P, 1], F32, tag="den")
        nc.vector.tensor_reduce(out=den[:], in_=cu[:], op=mybir.AluOpType.add,
                                axis=mybir.AxisListType.X)
        nc.vector.tensor_scalar_add(den[:], den[:], 1e-8)
        rden = rt.tile([P, 1], F32, tag="rden")
        nc.vector.reciprocal(rden[:], den[:])
        wloc = rt.tile([P, EL], F32, tag="wloc")
        nc.vector.tensor_scalar_mul(wloc[:], cu[:, 0:EL], rden[:, 0:1])
        sb = stk[:, ch * STK:(ch + 1) * STK]
        nc.vector.tensor_copy(sb[:, 0:EL], wloc[:])          # w hi (bf16)
        whif = rt.tile([P, EL], F32, tag="whif")
        nc.vector.tensor_copy(whif[:], sb[:, 0:EL])
        wres = rt.tile([P, EL], F32, tag="wres")
        nc.vector.tensor_sub(wres[:], wloc[:], whif[:])
        nc.vector.tensor_copy(sb[:, EL:2 * EL], wres[:])     # w lo (bf16)
        # capacity slots: masked_slot = (pref + runoff) * m6 - 1
        pf = ppB.tile([P, E], F32, tag="small")
        nc.tensor.matmul(pf[:], lhsT=ltri[:], rhs=m6[:], start=True, stop=True)
        s0 = rt.tile([P, E], F32, tag="s0")
        nc.vector.tensor_add(s0[:], pf[:], runoff[:])
        s1 = rt.tile([P, E], F32, tag="s1")
        nc.vector.tensor_mul(s1[:], s0[:], m6[:])
        nc.vector.tensor_scalar_sub(slotbuf[:, ch * E:(ch + 1) * E], s1[:], 1.0)
        # runoff = broadcast(row 127 of (pref + runoff))
        rb = ppB.tile([P, E], F32, tag="small")
        nc.tensor.matmul(rb[:], lhsT=r127[:], rhs=s0[:], start=True, stop=True)
        nc.vector.tensor_copy(runoff[:], rb[:])

    # ---- phase D: local experts (two groups of EGRP, combined per group) ----
    mets, yscs = {}, {}
    for e in range(EL):
        # one-hot slot matrices for this expert, all chunks
        me = ex.tile([P, NCH * C], BF16, tag="me", name=f"me{rep}_{e}")
        for ch in range(NCH):
            nc.vector.tensor_scalar(
                me[:, ch * C:(ch + 1) * C], iotac[:],
                slotbuf[:, ch * E + e: ch * E + e + 1],
                None, op0=mybir.AluOpType.is_equal)
        # slot weights: w[j] = sum_t me[t, j] * (whi + wlo)[t]
        ep = ppB.tile([C, STK], F32, tag="small")
        for ch in range(NCH):
            nc.tensor.matmul(ep[:], lhsT=me[:, ch * C:(ch + 1) * C],
                             rhs=stk[:, ch * STK:(ch + 1) * STK],
                             start=(ch == 0), stop=(ch == NCH - 1))
        wcol = ex.tile([C, 1], F32, tag="wcol")
        nc.vector.tensor_copy(wcol[:], ep[:, e:e + 1])
        nc.vector.tensor_add(wcol[:], wcol[:], ep[:, EL + e:EL + e + 1])
        # token gather on PE: xt[hk] = sum_tc xloc_chunk^T @ me_chunk
        xt = ex.tile([P, KH * P], BF16, tag="xt")
        for hk in range(KH):
            gp = ppA.tile([P, C], F32, tag="xg")
            for tch in range(NCH):
                nc.tensor.matmul(
                    gp[:], lhsT=xloc[:, tch * H + hk * P: tch * H + (hk + 1) * P],
                    rhs=me[:, tch * C:(tch + 1) * C],
                    start=(tch == 0), stop=(tch == NCH - 1))
            nc.vector.tensor_copy(xt[:, hk * P:(hk + 1) * P], gp[:])
        # Me^T for the combine
        met = cmb.tile([P, NCH * C], BF16, tag="met", name=f"met{rep}_{e}")
        for ch in range(NCH):
            tp = ppA.tile([P, P], BF16, tag="tr")
            nc.tensor.transpose(tp[:], me[:, ch * C:(ch + 1) * C], ident[:])
            nc.scalar.activation(met[:, ch * C:(ch + 1) * C], tp[:],
                                 AF.Copy)
        # layer 1 + swiglu, token-major [C, II]
        pg = ppA.tile([C, II], F32, tag="l1g")
        pu = ppA.tile([C, II], F32, tag="l1u")
        for kk in range(KH):
            nc.tensor.matmul(pg[:], lhsT=xt[:, kk * P:(kk + 1) * P],
                             rhs=wa[e][:, kk * II:(kk + 1) * II],
                             start=(kk == 0), stop=(kk == KH - 1))
        for kk in range(KH):
            nc.tensor.matmul(
                pu[:], lhsT=xt[:, kk * P:(kk + 1) * P],
                rhs=wa[e][:, 4096 + kk * II: 4096 + (kk + 1) * II],
                start=(kk == 0), stop=(kk == KH - 1))
        sa = ex.tile([C, II], F32, tag="sact")
        nc.scalar.activation(sa[:], pg[:], AF.Silu)
        h2 = ex.tile([C, II], BF16, tag="h2")
        nc.vector.tensor_mul(h2[:], sa[:], pu[:])
        # transpose h2 -> [II, C]
        h2t = ex.tile([P, 4 * C], BF16, tag="h2t")
        for kk in range(4):
            tp = ppA.tile([P, P], BF16, tag="tr")
            nc.tensor.transpose(tp[:], h2[:, kk * P:(kk + 1) * P], ident[:])
            nc.vector.tensor_copy(h2t[:, kk * P:(kk + 1) * P], tp[:])
        # layer 3: y = h2 @ Wd^T, scaled by slot weight, cast bf16
        ysc = cmb.tile([C, H], BF16, tag="ysc", name=f"ysc{rep}_{e}")
        for hh in range(2):
            yp = ppB.tile([C, 512], F32, tag="l3")
            for kk in range(4):
                nc.tensor.matmul(
                    yp[:], lhsT=h2t[:, kk * P:(kk + 1) * P],
                    rhs=wd[e][:, kk * H + hh * 512:
                              kk * H + (hh + 1) * 512],
                    start=(kk == 0), stop=(kk == 3))
            nc.scalar.activation(ysc[:, hh * 512:(hh + 1) * 512],
                                 yp[:], AF.Copy, bias=0.0, scale=wcol[:, 0:1])
        mets[e], yscs[e] = met, ysc

        # combine after each group of EGRP experts
        if e % EGRP == EGRP - 1:
            grp = list(range(e - EGRP + 1, e + 1))
            last = e == EL - 1
            for tch in range(NCH):
                for hh in range(2):
                    cp = ppB.tile([P, 512], F32, tag="l3")
                    for gi, ge in enumerate(grp):
                        nc.tensor.matmul(
                            cp[:], lhsT=mets[ge][:, tch * C:(tch + 1) * C],
                            rhs=yscs[ge][:, hh * 512:(hh + 1) * 512],
                            start=(gi == 0), stop=(gi == EGRP - 1))
                    ysl = yshf[:, tch * H + hh * 512: tch * H + (hh + 1) * 512]
                    nc.vector.tensor_add(ysl, ysl, cp[:])
                if last:
                    nc.sync.dma_start(
                        out_d[tch * P:(tch + 1) * P, :],
                        yshf[:, tch * H:(tch + 1) * H])


_PROG = None


def _pack(a):
    """[KH*P, F] -> [P, KH*F] with chunk kk at columns kk*F:(kk+1)*F."""
    kh = a.shape[0] // P
    return np.ascontiguousarray(
        a.reshape(kh, P, -1).transpose(1, 0, 2).reshape(P, -1))


def _prep_core_inputs(c, x, gate_w, gate_bias, eg_w, eu_w, ed_w, sg_w, su_w, sd_w):
    perm = [c] + [g for g in range(NC) if g != c]
    eperm = np.concatenate([np.arange(g * 8, g * 8 + 8) for g in perm])

    xT = np.ascontiguousarray(x.T)                       # [H, T]
    gT = np.ascontiguousarray(gate_w[eperm].T)           # [H, E]

    wbloba = np.empty((EL, P, 8192), BF)
    wblobd = np.empty((EL, P, 4096), BF)
    for e in range(EL):
        ge = c * 8 + e
        wbloba[e, :, 0:4096] = _pack(eg_w[ge].T.astype(BF))
        wbloba[e, :, 4096:8192] = _pack(eu_w[ge].T.astype(BF))
        wblobd[e] = _pack(ed_w[ge].T.astype(BF))

    sl = slice(c * P, (c + 1) * P)
    shw = np.empty((P, 3072), BF)
    shw[:, 0:1024] = _pack(sg_w[sl].T.astype(BF))
    shw[:, 1024:2048] = _pack(su_w[sl].T.astype(BF))
    shw[:, 2048:3072] = np.ascontiguousarray(sd_w[:, sl].T).astype(BF)

    return {
        "xtf": _pack(xT), "xthi": _pack(xT.astype(BF)),
        "xloc": _pack(x.astype(BF)),
        "gtf": _pack(gT),
        "wbloba": wbloba, "wblobd": wblobd, "shw": shw,
        "ltri": np.triu(np.ones((P, P), np.float32)),
        "r127": np.concatenate([np.zeros((127, P), np.float32),
                                np.ones((1, P), np.float32)]),
        "iotac": np.broadcast_to(np.arange(C, dtype=np.float32), (P, C)).copy(),
        "biasbc": np.broadcast_to(
            gate_bias[eperm].astype(np.float32), (P, E)).copy(),
    }


def kernel(hidden_states, gate_w, gate_bias, eg_w, eu_w, ed_w, sg_w, su_w, sd_w):
    global _PROG
    if _PROG is None:
        _PROG = _build_program()
    nc = _PROG

    x = np.asarray(hidden_states, np.float32).reshape(T, H)
    args = [np.asarray(a, np.float32) for a in
            (gate_w, gate_bias, eg_w, eu_w, ed_w, sg_w, su_w, sd_w)]
    in_maps = [_prep_core_inputs(c, x, *args) for c in range(NC)]
    res = run_bass_kernel_spmd(nc, in_maps, list(range(NC)))
    out = np.zeros((T, H), np.float32)
    for c in range(NC):
        out += res.results[c]["out"]
    return out.reshape(1, T, H)

